# revision 9
# baseline (speedup 1.0000x reference)
"""Trainium2 Bass kernel for nn_Attn (B=32, S=4096, H=1024, D=2*H=2048).

Reference computation:
    tmp      = einsum("bsd,hd->bsh", encoder_outputs, W) + b      # [B,S,H]
    energies = einsum("bh,bsh->bs", hidden, tmp)                  # [B,S]
    attn     = softmax(energies, axis=-1)[:, None, :]             # [B,1,S]

Key reassociation (exact in real arithmetic):
    energies[b,s] = enc[b,s,:] . v[b,:] + (hidden[b] . bias)
    with v[b,:] = hidden[b,:] @ W        # [B, D]
The bias term is constant over s, so it cancels inside softmax and is
dropped entirely.  This turns a 550-GFLOP dense matmul problem into a
memory-bound weighted-reduction stream over the 1 GiB encoder_outputs.

Sharding: data-parallel over batch across 8 cores (4 batches/core),
W replicated.  (A tensor-parallel W-shard + 32 KiB AllToAll for v was
tried and is ~20 us better on paper, but the NRT collective's prelude
barrier surfaces ~30-50 us of cross-core NEFF launch skew, which eats
the win -- so no collectives.)  Per core:
  1. W streams through a 4-deep k-tile pool at the head of the sync
     HWDGE queue (back-to-back DMAs; the 4-deep pipeline keeps the
     FIFO from stalling the enc stream behind it) and v = hidden @ W
     accumulates on TensorE as tiles land,
  2. v[b] rows move to partition-base-0 tiles (batch 0 straight out of
     PSUM partition 0 via ScalarE -- fastest path, it gates the DVE
     start; batches 1-3 via SBUF->SBUF DMA, needed much later) and are
     broadcast to 128 partitions with a rank-1 TensorE matmul
     (ones[1,128] outer v[b]) -- no DRAM roundtrip,
  3. enc tiles [128 s-partitions x 2 x 2048 d] stream on the sync
     queue (7 x 2 MiB buffers) and reduce on DVE with fused
     scalar_tensor_tensor (out = in0 * in1, accum_out = row-sum)
     against the broadcast v,
  4. softmax runs per batch as soon as that batch's stream finishes
     (overlapped with later batches' streaming), entirely in the
     [128, 32] energy layout: per-partition max/exp/sum on DVE/ScalarE,
     cross-partition max/sum via TensorE transpose-with-identity, and
     scalar broadcasts via ones-matmul,
  5. each batch's attn [128, 32] tile DMAs straight to out[b] on the
     SWDGE queue (keeps the sync queue pure enc streaming).
"""

import numpy as np

import concourse.bacc as bacc
import concourse.tile as tile
from concourse import mybir
from concourse.bass_utils import run_bass_kernel_spmd

F32 = mybir.dt.float32

B, S, H, D = 32, 4096, 1024, 2048
NCORES = 8
BL = B // NCORES          # batches per core = 4
KT = H // 128             # hidden k-tiles = 8
NJ = D // 512             # 512-wide N chunks in D = 4
SJ = 2                    # s-rows per partition per streamed DMA chunk
NQ = S // (128 * SJ)      # streamed DMA chunks per batch = 16
SCOLS = S // 128          # energy columns per partition = 32
STREAM_BUFS = 7
W_BUFS = 4


def build_bass():
    nc = bacc.Bacc()
    # hT[p, k*BL + m] = hidden_loc[m, k*128 + p]  (per-core batches)
    hT = nc.dram_tensor("hT", [128, KT * BL], F32, kind="ExternalInput")
    W = nc.dram_tensor("W", [H, D], F32, kind="ExternalInput")
    enc = nc.dram_tensor("enc", [BL, S, D], F32, kind="ExternalInput")
    ident = nc.dram_tensor("ident", [128, 128], F32, kind="ExternalInput")
    ones = nc.dram_tensor("ones", [1, 128], F32, kind="ExternalInput")
    out = nc.dram_tensor("out", [BL, S], F32, kind="ExternalOutput")

    with tile.TileContext(nc) as tc:
        with (
            tc.tile_pool(name="persist", bufs=1) as persist,
            tc.tile_pool(name="wpool", bufs=W_BUFS) as wpool,
            tc.tile_pool(name="vrpool", bufs=1) as vrpool,
            tc.tile_pool(name="stream", bufs=STREAM_BUFS) as stream,
            tc.tile_pool(name="psum_v", bufs=1, space="PSUM") as psum_v_pool,
            tc.tile_pool(name="psum_b", bufs=2, space="PSUM") as psum_b_pool,
            tc.tile_pool(name="psum_s", bufs=1, space="PSUM") as psum_s_pool,
        ):
            # ---- small loads first on the sync queue ----
            hT_sb = persist.tile([128, KT * BL], F32, tag="hT")
            nc.sync.dma_start(out=hT_sb, in_=hT[:, :])
            ident_sb = persist.tile([128, 128], F32, tag="ident")
            nc.sync.dma_start(out=ident_sb, in_=ident[:, :])
            ones_sb = persist.tile([1, 128], F32, tag="ones")
            nc.sync.dma_start(out=ones_sb, in_=ones[:, :])

            # ---- dummy matmul to absorb the PE sequencer's ~8 us first-
            # dispatch latency while W is still loading ----
            warm = psum_s_pool.tile([1, 128], F32, tag="tr")
            nc.tensor.matmul(
                warm, ones_sb[:, 0:1], ones_sb, start=True, stop=True
            )

            # ---- v = hidden_loc @ W -> psum [BL, D], W cycled via pool ----
            psv = psum_v_pool.tile([BL, D], F32, tag="psv")
            for k in range(KT):
                wt = wpool.tile([128, D], F32, tag="w", name=f"w{k}")
                nc.sync.dma_start(out=wt, in_=W[k * 128:(k + 1) * 128, :])
                for j in range(NJ):
                    nc.tensor.matmul(
                        psv[:, j * 512:(j + 1) * 512],
                        hT_sb[:, k * BL:(k + 1) * BL],
                        wt[:, j * 512:(j + 1) * 512],
                        start=(k == 0),
                        stop=(k == KT - 1),
                    )
            # batch 0 fast path: partition 0 of PSUM is legal for ACT, so
            # copy the v row straight out of psv -- no v_sb / SWDGE hops.
            vr0 = persist.tile([1, D], F32, tag="vr0")
            nc.scalar.copy(out=vr0, in_=psv[0:1, :])

            # batches 1-3 are needed much later: go via v_sb + SBUF->SBUF
            # DMA (engines can't touch partition offsets 1..3, DMA can).
            v_sb = persist.tile([BL, D], F32, tag="vsb")
            nc.scalar.copy(out=v_sb, in_=psv)

            # ---- per batch: broadcast v[b] to 128 partitions via rank-1
            # matmul ----
            v_bc = []
            for b in range(BL):
                if b == 0:
                    vr = vr0
                else:
                    vr = vrpool.tile([1, D], F32, tag="vr", name=f"vr{b}")
                    nc.gpsimd.dma_start(out=vr, in_=v_sb[b:b + 1, :])
                vb = persist.tile([128, D], F32, tag=f"vb{b}", name=f"vb{b}")
                for j in range(NJ):
                    pb = psum_b_pool.tile([128, 512], F32, tag="pbc")
                    nc.tensor.matmul(
                        pb,
                        ones_sb,
                        vr[:, j * 512:(j + 1) * 512],
                        start=True,
                        stop=True,
                    )
                    # alternate engines so consecutive chunk copies overlap
                    if j % 2 == 0:
                        nc.vector.tensor_copy(
                            out=vb[:, j * 512:(j + 1) * 512], in_=pb
                        )
                    else:
                        nc.scalar.copy(out=vb[:, j * 512:(j + 1) * 512], in_=pb)
                v_bc.append(vb)

            # ---- stream enc, fused multiply + row-reduce on DVE ----
            # s = p*SCOLS + q*SJ + j   (p = partition, column c = q*SJ + j)
            enc_r = enc[:, :, :].rearrange(
                "b (p q j) d -> b q p j d", p=128, q=NQ, j=SJ
            )
            e_tiles = [
                persist.tile([128, SCOLS], F32, tag=f"e{b}", name=f"e{b}")
                for b in range(BL)
            ]
            for b in range(BL):
                for q in range(NQ):
                    t = stream.tile([128, SJ, D], F32, tag="enc", name="enc_t")
                    nc.sync.dma_start(out=t, in_=enc_r[b, q])
                    for j in range(SJ):
                        # Fused multiply + add-reduce on DVE in one pass:
                        # out = (in0 * 1.0) * in1, accum_out = sum(out).
                        # out aliases in0 (the product is dead after the
                        # reduce).  NB: tensor_tensor_reduce wedges the device
                        # on this runtime path; scalar_tensor_tensor is the
                        # plain TENSOR_SCALAR_PTR ISA op and works.
                        nc.vector.scalar_tensor_tensor(
                            out=t[:, j, :],
                            in0=t[:, j, :],
                            scalar=1.0,
                            in1=v_bc[b],
                            op0=mybir.AluOpType.mult,
                            op1=mybir.AluOpType.mult,
                            accum_out=e_tiles[b][:, q * SJ + j:q * SJ + j + 1],
                        )

                # ---- per-batch softmax in the [128, SCOLS] layout,
                # overlapped with the next batch's streaming ----
                e = e_tiles[b]
                m_p = persist.tile([128, 1], F32, tag=f"mp{b}")
                nc.vector.tensor_reduce(
                    out=m_p, in_=e, axis=mybir.AxisListType.X,
                    op=mybir.AluOpType.max,
                )
                nm_p = persist.tile([128, 1], F32, tag=f"nmp{b}")
                nc.scalar.mul(out=nm_p, in_=m_p, mul=-1.0)
                s_p = persist.tile([128, 1], F32, tag=f"sp{b}")
                # e <- exp(e - m_p), s_p = row sums
                nc.scalar.activation(
                    out=e, in_=e,
                    func=mybir.ActivationFunctionType.Exp,
                    bias=nm_p, scale=1.0, accum_out=s_p,
                )
                # M = max_p m_p  (transpose via PE, reduce on DVE)
                mT = psum_s_pool.tile([1, 128], F32, tag="tr")
                nc.tensor.transpose(mT, m_p, ident_sb)
                mx = persist.tile([1, 1], F32, tag=f"mx{b}")
                nc.vector.tensor_reduce(
                    out=mx, in_=mT, axis=mybir.AxisListType.X,
                    op=mybir.AluOpType.max,
                )
                # -M broadcast to 128 partitions
                nmx = persist.tile([1, 1], F32, tag=f"nmx{b}")
                nc.scalar.mul(out=nmx, in_=mx, mul=-1.0)
                negMb = psum_s_pool.tile([128, 1], F32, tag="bc1")
                nc.tensor.matmul(
                    negMb, ones_sb, nmx[0:1, 0:1], start=True, stop=True
                )
                # w_p = exp(m_p - M)
                w_p = persist.tile([128, 1], F32, tag=f"wp{b}")
                nc.scalar.activation(
                    out=w_p, in_=negMb,
                    func=mybir.ActivationFunctionType.Exp,
                    bias=m_p, scale=1.0,
                )
                # Sw_p = s_p * w_p ; D = sum_p Sw_p
                sw_p = persist.tile([128, 1], F32, tag=f"swp{b}")
                nc.vector.scalar_tensor_tensor(
                    out=sw_p, in0=s_p, scalar=1.0, in1=w_p,
                    op0=mybir.AluOpType.mult, op1=mybir.AluOpType.mult,
                )
                swT = psum_s_pool.tile([1, 128], F32, tag="tr")
                nc.tensor.transpose(swT, sw_p, ident_sb)
                dsum = persist.tile([1, 1], F32, tag=f"ds{b}")
                nc.vector.tensor_reduce(
                    out=dsum, in_=swT, axis=mybir.AxisListType.X,
                    op=mybir.AluOpType.add,
                )
                rden = persist.tile([1, 1], F32, tag=f"rd{b}")
                nc.vector.reciprocal(out=rden, in_=dsum)
                rb = psum_s_pool.tile([128, 1], F32, tag="bc1")
                nc.tensor.matmul(
                    rb, ones_sb, rden[0:1, 0:1], start=True, stop=True
                )
                # f_p = w_p * (1/D) ; attn = e * f_p
                f_p = persist.tile([128, 1], F32, tag=f"fp{b}")
                nc.vector.scalar_tensor_tensor(
                    out=f_p, in0=w_p, scalar=1.0, in1=rb,
                    op0=mybir.AluOpType.mult, op1=mybir.AluOpType.mult,
                )
                nc.vector.tensor_scalar_mul(e, e, f_p)
                # out[b, p*SCOLS + c] = e[p, c]; SWDGE queue keeps the sync
                # queue pure enc streaming.
                nc.gpsimd.dma_start(out=out[b:b + 1, :], in_=e[:, :])

    nc.compile()
    return nc


_NC_CACHE = None


def _get_nc():
    global _NC_CACHE
    if _NC_CACHE is None:
        _NC_CACHE = build_bass()
    return _NC_CACHE


def _make_in_maps(hidden, encoder_outputs, W):
    hidden = np.asarray(hidden, dtype=np.float32)
    encoder_outputs = np.asarray(encoder_outputs, dtype=np.float32)
    W = np.ascontiguousarray(np.asarray(W, dtype=np.float32))
    ident = np.eye(128, dtype=np.float32)
    ones = np.ones((1, 128), np.float32)
    in_maps = []
    for c in range(NCORES):
        hs = hidden[c * BL:(c + 1) * BL]                       # [BL, H]
        hT = np.ascontiguousarray(
            hs.T.reshape(KT, 128, BL).transpose(1, 0, 2).reshape(128, KT * BL)
        )
        in_maps.append({
            "hT": hT,
            "W": W,
            "enc": np.ascontiguousarray(encoder_outputs[c * BL:(c + 1) * BL]),
            "ident": ident,
            "ones": ones,
        })
    return in_maps


def run_device(hidden, encoder_outputs, W, trace=False, **spmd_kwargs):
    nc = _get_nc()
    in_maps = _make_in_maps(hidden, encoder_outputs, W)
    res = run_bass_kernel_spmd(
        nc, in_maps, core_ids=list(range(NCORES)), trace=trace, **spmd_kwargs
    )
    outs = np.concatenate([r["out"] for r in res.results], axis=0)  # [B, S]
    return outs[:, None, :].astype(np.float32), res


def kernel(hidden, encoder_outputs, W, b):
    # `b` (the Linear bias) shifts every energy in a row equally
    # (hidden[b].bias, independent of s), so it cancels in the softmax.
    out, _ = run_device(hidden, encoder_outputs, W)
    return out


# revision 11
# speedup vs baseline: 1.2821x; 1.2821x over previous
"""Trainium2 Bass kernel for nn_Attn (B=32, S=4096, H=1024, D=2*H=2048).

Reference computation:
    tmp      = einsum("bsd,hd->bsh", encoder_outputs, W) + b      # [B,S,H]
    energies = einsum("bh,bsh->bs", hidden, tmp)                  # [B,S]
    attn     = softmax(energies, axis=-1)[:, None, :]             # [B,1,S]

Key reassociation (exact in real arithmetic):
    energies[b,s] = enc[b,s,:] . v[b,:] + (hidden[b] . bias)
    with v[b,:] = hidden[b,:] @ W        # [B, D]
The bias term is constant over s, so it cancels inside softmax and is
dropped entirely.  This turns a 550-GFLOP dense matmul problem into a
memory-bound weighted-reduction stream over the 1 GiB encoder_outputs.

Sharding: data-parallel over batch across 8 cores (4 batches/core),
W replicated.  (A tensor-parallel W-shard + 32 KiB AllToAll for v was
tried and is ~20 us better on paper, but the NRT collective's prelude
barrier surfaces ~30-50 us of cross-core NEFF launch skew, which eats
the win -- so no collectives.)  Per core:
  1. W streams through a 4-deep k-tile pool at the head of the sync
     HWDGE queue (back-to-back DMAs; the 4-deep pipeline keeps the
     FIFO from stalling the enc stream behind it) and v = hidden @ W
     accumulates on TensorE as tiles land,
  2. v[b] rows move to partition-base-0 tiles (batch 0 straight out of
     PSUM partition 0 via ScalarE -- fastest path, it gates the DVE
     start; batches 1-3 via SBUF->SBUF DMA, needed much later) and are
     broadcast to 128 partitions with a rank-1 TensorE matmul
     (ones[1,128] outer v[b]) -- no DRAM roundtrip,
  3. enc tiles [128 s-partitions x 2 x 2048 d] stream on the sync
     queue (7 x 2 MiB buffers) and reduce on DVE with fused
     scalar_tensor_tensor (out = in0 * in1, accum_out = row-sum)
     against the broadcast v,
  4. softmax runs per batch as soon as that batch's stream finishes
     (overlapped with later batches' streaming), entirely in the
     [128, 32] energy layout: per-partition max/exp/sum on DVE/ScalarE,
     cross-partition max/sum via TensorE transpose-with-identity, and
     scalar broadcasts via ones-matmul,
  5. each batch's attn [128, 32] tile DMAs straight to out[b] on the
     SWDGE queue (keeps the sync queue pure enc streaming).
"""

import numpy as np

import concourse.bacc as bacc
import concourse.tile as tile
from concourse import mybir
from concourse.bass_utils import run_bass_kernel_spmd

F32 = mybir.dt.float32

B, S, H, D = 32, 4096, 1024, 2048
NCORES = 8
BL = B // NCORES          # batches per core = 4
KT = H // 128             # hidden k-tiles = 8
NJ = D // 512             # 512-wide N chunks in D = 4
SJ = 2                    # s-rows per partition per streamed DMA chunk
NQ = S // (128 * SJ)      # streamed DMA chunks per batch = 16
SCOLS = S // 128          # energy columns per partition = 32
STREAM_BUFS = 7
W_BUFS = 4


def build_bass():
    nc = bacc.Bacc()
    # hT[p, k*BL + m] = hidden_loc[m, k*128 + p]  (per-core batches)
    hT = nc.dram_tensor("hT", [128, KT * BL], F32, kind="ExternalInput")
    W = nc.dram_tensor("W", [H, D], F32, kind="ExternalInput")
    enc = nc.dram_tensor("enc", [BL, S, D], F32, kind="ExternalInput")
    ident = nc.dram_tensor("ident", [128, 128], F32, kind="ExternalInput")
    ones = nc.dram_tensor("ones", [1, 128], F32, kind="ExternalInput")
    out = nc.dram_tensor("out", [BL, S], F32, kind="ExternalOutput")

    with tile.TileContext(nc) as tc:
        with (
            tc.tile_pool(name="persist", bufs=1) as persist,
            tc.tile_pool(name="wpool", bufs=W_BUFS) as wpool,
            tc.tile_pool(name="vrpool", bufs=1) as vrpool,
            tc.tile_pool(name="stream", bufs=STREAM_BUFS) as stream,
            tc.tile_pool(name="psum_v", bufs=1, space="PSUM") as psum_v_pool,
            tc.tile_pool(name="psum_b", bufs=2, space="PSUM") as psum_b_pool,
            tc.tile_pool(name="psum_s", bufs=1, space="PSUM") as psum_s_pool,
        ):
            # ---- small loads first on the sync queue ----
            hT_sb = persist.tile([128, KT * BL], F32, tag="hT")
            nc.sync.dma_start(out=hT_sb, in_=hT[:, :])
            ident_sb = persist.tile([128, 128], F32, tag="ident")
            nc.sync.dma_start(out=ident_sb, in_=ident[:, :])
            ones_sb = persist.tile([1, 128], F32, tag="ones")
            nc.sync.dma_start(out=ones_sb, in_=ones[:, :])

            # ---- dummy matmul to absorb the PE sequencer's ~8 us first-
            # dispatch latency while W is still loading ----
            warm = psum_s_pool.tile([1, 128], F32, tag="tr")
            nc.tensor.matmul(
                warm, ones_sb[:, 0:1], ones_sb, start=True, stop=True
            )

            # ---- v = hidden_loc @ W -> psum [BL, D], W cycled via pool ----
            psv = psum_v_pool.tile([BL, D], F32, tag="psv")
            for k in range(KT):
                wt = wpool.tile([128, D], F32, tag="w", name=f"w{k}")
                nc.sync.dma_start(out=wt, in_=W[k * 128:(k + 1) * 128, :])
                for j in range(NJ):
                    nc.tensor.matmul(
                        psv[:, j * 512:(j + 1) * 512],
                        hT_sb[:, k * BL:(k + 1) * BL],
                        wt[:, j * 512:(j + 1) * 512],
                        start=(k == 0),
                        stop=(k == KT - 1),
                    )
            # batch 0 fast path: partition 0 of PSUM is legal for ACT, so
            # copy the v row straight out of psv -- no v_sb / SWDGE hops.
            # Chunked: copy j waits only on its own chunk's k=7 stop, so it
            # overlaps the remaining psv matmuls instead of waiting for all.
            vr0 = persist.tile([1, D], F32, tag="vr0")
            for j in range(NJ):
                nc.scalar.copy(
                    out=vr0[:, j * 512:(j + 1) * 512],
                    in_=psv[0:1, j * 512:(j + 1) * 512],
                )

            # batches 1-3 are needed much later: go via v_sb + SBUF->SBUF
            # DMA (engines can't touch partition offsets 1..3, DMA can).
            # The v_sb copy is issued after the b0 broadcast block so it
            # can't delay the DVE start on the ACT queue.
            v_sb = persist.tile([BL, D], F32, tag="vsb")

            # ---- per batch: broadcast v[b] to 128 partitions via rank-1
            # matmul ----
            v_bc = []
            for b in range(BL):
                if b == 0:
                    vr = vr0
                else:
                    if b == 1:
                        nc.scalar.copy(out=v_sb, in_=psv)
                    vr = vrpool.tile([1, D], F32, tag="vr", name=f"vr{b}")
                    nc.gpsimd.dma_start(out=vr, in_=v_sb[b:b + 1, :])
                vb = persist.tile([128, D], F32, tag=f"vb{b}", name=f"vb{b}")
                for j in range(NJ):
                    pb = psum_b_pool.tile([128, 512], F32, tag="pbc")
                    nc.tensor.matmul(
                        pb,
                        ones_sb,
                        vr[:, j * 512:(j + 1) * 512],
                        start=True,
                        stop=True,
                    )
                    # b0 alternates engines (it gates the DVE start);
                    # later batches stay off DVE so they can't stall the
                    # streaming reduction with PE-sem waits.
                    if b == 0 and j % 2 == 0:
                        nc.vector.tensor_copy(
                            out=vb[:, j * 512:(j + 1) * 512], in_=pb
                        )
                    else:
                        nc.scalar.copy(out=vb[:, j * 512:(j + 1) * 512], in_=pb)
                v_bc.append(vb)

            # ---- stream enc, fused multiply + row-reduce on DVE ----
            # s = p*SCOLS + q*SJ + j   (p = partition, column c = q*SJ + j)
            enc_r = enc[:, :, :].rearrange(
                "b (p q j) d -> b q p j d", p=128, q=NQ, j=SJ
            )
            e_tiles = [
                persist.tile([128, SCOLS], F32, tag=f"e{b}", name=f"e{b}")
                for b in range(BL)
            ]
            for b in range(BL):
                for q in range(NQ):
                    t = stream.tile([128, SJ, D], F32, tag="enc", name="enc_t")
                    nc.sync.dma_start(out=t, in_=enc_r[b, q])
                    for j in range(SJ):
                        # Fused multiply + add-reduce in one pass:
                        # out = (in0 * 1.0) * in1, accum_out = sum(out).
                        # out aliases in0 (the product is dead after the
                        # reduce).  NB: tensor_tensor_reduce wedges the device
                        # on this runtime path; scalar_tensor_tensor is the
                        # plain TENSOR_SCALAR_PTR ISA op and works.
                        nc.vector.scalar_tensor_tensor(
                            out=t[:, j, :],
                            in0=t[:, j, :],
                            scalar=1.0,
                            in1=v_bc[b],
                            op0=mybir.AluOpType.mult,
                            op1=mybir.AluOpType.mult,
                            accum_out=e_tiles[b][:, q * SJ + j:q * SJ + j + 1],
                        )

                # ---- per-batch softmax in the [128, SCOLS] layout,
                # overlapped with the next batch's streaming ----
                e = e_tiles[b]
                m_p = persist.tile([128, 1], F32, tag=f"mp{b}")
                nc.vector.tensor_reduce(
                    out=m_p, in_=e, axis=mybir.AxisListType.X,
                    op=mybir.AluOpType.max,
                )
                nm_p = persist.tile([128, 1], F32, tag=f"nmp{b}")
                nc.scalar.mul(out=nm_p, in_=m_p, mul=-1.0)
                s_p = persist.tile([128, 1], F32, tag=f"sp{b}")
                # e <- exp(e - m_p), s_p = row sums
                nc.scalar.activation(
                    out=e, in_=e,
                    func=mybir.ActivationFunctionType.Exp,
                    bias=nm_p, scale=1.0, accum_out=s_p,
                )
                # M = max_p m_p  (transpose via PE, reduce on DVE)
                mT = psum_s_pool.tile([1, 128], F32, tag="tr")
                nc.tensor.transpose(mT, m_p, ident_sb)
                mx = persist.tile([1, 1], F32, tag=f"mx{b}")
                nc.vector.tensor_reduce(
                    out=mx, in_=mT, axis=mybir.AxisListType.X,
                    op=mybir.AluOpType.max,
                )
                # -M broadcast to 128 partitions
                nmx = persist.tile([1, 1], F32, tag=f"nmx{b}")
                nc.scalar.mul(out=nmx, in_=mx, mul=-1.0)
                negMb = psum_s_pool.tile([128, 1], F32, tag="bc1")
                nc.tensor.matmul(
                    negMb, ones_sb, nmx[0:1, 0:1], start=True, stop=True
                )
                # w_p = exp(m_p - M)
                w_p = persist.tile([128, 1], F32, tag=f"wp{b}")
                nc.scalar.activation(
                    out=w_p, in_=negMb,
                    func=mybir.ActivationFunctionType.Exp,
                    bias=m_p, scale=1.0,
                )
                # Sw_p = s_p * w_p ; D = sum_p Sw_p
                sw_p = persist.tile([128, 1], F32, tag=f"swp{b}")
                nc.vector.scalar_tensor_tensor(
                    out=sw_p, in0=s_p, scalar=1.0, in1=w_p,
                    op0=mybir.AluOpType.mult, op1=mybir.AluOpType.mult,
                )
                swT = psum_s_pool.tile([1, 128], F32, tag="tr")
                nc.tensor.transpose(swT, sw_p, ident_sb)
                dsum = persist.tile([1, 1], F32, tag=f"ds{b}")
                nc.vector.tensor_reduce(
                    out=dsum, in_=swT, axis=mybir.AxisListType.X,
                    op=mybir.AluOpType.add,
                )
                rden = persist.tile([1, 1], F32, tag=f"rd{b}")
                nc.vector.reciprocal(out=rden, in_=dsum)
                rb = psum_s_pool.tile([128, 1], F32, tag="bc1")
                nc.tensor.matmul(
                    rb, ones_sb, rden[0:1, 0:1], start=True, stop=True
                )
                # f_p = w_p * (1/D) ; attn = e * f_p
                f_p = persist.tile([128, 1], F32, tag=f"fp{b}")
                nc.vector.scalar_tensor_tensor(
                    out=f_p, in0=w_p, scalar=1.0, in1=rb,
                    op0=mybir.AluOpType.mult, op1=mybir.AluOpType.mult,
                )
                nc.vector.tensor_scalar_mul(e, e, f_p)
                # out[b, p*SCOLS + c] = e[p, c]; SWDGE queue keeps the sync
                # queue pure enc streaming.
                nc.gpsimd.dma_start(out=out[b:b + 1, :], in_=e[:, :])

    nc.compile()
    return nc


_NC_CACHE = None


def _get_nc():
    global _NC_CACHE
    if _NC_CACHE is None:
        _NC_CACHE = build_bass()
    return _NC_CACHE


def _make_in_maps(hidden, encoder_outputs, W):
    hidden = np.asarray(hidden, dtype=np.float32)
    encoder_outputs = np.asarray(encoder_outputs, dtype=np.float32)
    W = np.ascontiguousarray(np.asarray(W, dtype=np.float32))
    ident = np.eye(128, dtype=np.float32)
    ones = np.ones((1, 128), np.float32)
    in_maps = []
    for c in range(NCORES):
        hs = hidden[c * BL:(c + 1) * BL]                       # [BL, H]
        hT = np.ascontiguousarray(
            hs.T.reshape(KT, 128, BL).transpose(1, 0, 2).reshape(128, KT * BL)
        )
        in_maps.append({
            "hT": hT,
            "W": W,
            "enc": np.ascontiguousarray(encoder_outputs[c * BL:(c + 1) * BL]),
            "ident": ident,
            "ones": ones,
        })
    return in_maps


def run_device(hidden, encoder_outputs, W, trace=False, **spmd_kwargs):
    nc = _get_nc()
    in_maps = _make_in_maps(hidden, encoder_outputs, W)
    res = run_bass_kernel_spmd(
        nc, in_maps, core_ids=list(range(NCORES)), trace=trace, **spmd_kwargs
    )
    outs = np.concatenate([r["out"] for r in res.results], axis=0)  # [B, S]
    return outs[:, None, :].astype(np.float32), res


def kernel(hidden, encoder_outputs, W, b):
    # `b` (the Linear bias) shifts every energy in a row equally
    # (hidden[b].bias, independent of s), so it cancels in the softmax.
    out, _ = run_device(hidden, encoder_outputs, W)
    return out


# revision 12
# speedup vs baseline: 1.2911x; 1.0070x over previous
"""Trainium2 Bass kernel for nn_Attn (B=32, S=4096, H=1024, D=2*H=2048).

Reference computation:
    tmp      = einsum("bsd,hd->bsh", encoder_outputs, W) + b      # [B,S,H]
    energies = einsum("bh,bsh->bs", hidden, tmp)                  # [B,S]
    attn     = softmax(energies, axis=-1)[:, None, :]             # [B,1,S]

Key reassociation (exact in real arithmetic):
    energies[b,s] = enc[b,s,:] . v[b,:] + (hidden[b] . bias)
    with v[b,:] = hidden[b,:] @ W        # [B, D]
The bias term is constant over s, so it cancels inside softmax and is
dropped entirely.  This turns a 550-GFLOP dense matmul problem into a
memory-bound weighted-reduction stream over the 1 GiB encoder_outputs.

Sharding: data-parallel over batch across 8 cores (4 batches/core),
W replicated.  (A tensor-parallel W-shard + 32 KiB AllToAll for v was
tried and is ~20 us better on paper, but the NRT collective's prelude
barrier surfaces ~30-50 us of cross-core NEFF launch skew, which eats
the win -- so no collectives.)  Per core:
  1. W streams through a 4-deep k-tile pool at the head of the sync
     HWDGE queue (back-to-back DMAs; the 4-deep pipeline keeps the
     FIFO from stalling the enc stream behind it) and v = hidden @ W
     accumulates on TensorE as tiles land,
  2. v[b] rows move to partition-base-0 tiles (batch 0 straight out of
     PSUM partition 0 via ScalarE -- fastest path, it gates the DVE
     start; batches 1-3 via SBUF->SBUF DMA, needed much later) and are
     broadcast to 128 partitions with a rank-1 TensorE matmul
     (ones[1,128] outer v[b]) -- no DRAM roundtrip,
  3. enc tiles [128 s-partitions x 2 x 2048 d] stream on the sync
     queue (7 x 2 MiB buffers) and reduce on DVE with fused
     scalar_tensor_tensor (out = in0 * in1, accum_out = row-sum)
     against the broadcast v,
  4. softmax runs per batch as soon as that batch's stream finishes
     (overlapped with later batches' streaming), entirely in the
     [128, 32] energy layout: per-partition max/exp/sum on DVE/ScalarE,
     cross-partition max/sum via TensorE transpose-with-identity, and
     scalar broadcasts via ones-matmul,
  5. each batch's attn [128, 32] tile DMAs straight to out[b] on the
     SWDGE queue (keeps the sync queue pure enc streaming).
"""

import numpy as np

import concourse.bacc as bacc
import concourse.tile as tile
from concourse import mybir
from concourse.bass_utils import run_bass_kernel_spmd

F32 = mybir.dt.float32

B, S, H, D = 32, 4096, 1024, 2048
NCORES = 8
BL = B // NCORES          # batches per core = 4
KT = H // 128             # hidden k-tiles = 8
NJ = D // 512             # 512-wide N chunks in D = 4
SJ = 2                    # s-rows per partition per streamed DMA chunk
NQ = S // (128 * SJ)      # streamed DMA chunks per batch = 16
SCOLS = S // 128          # energy columns per partition = 32
STREAM_BUFS = 7
W_BUFS = 4


def build_bass():
    nc = bacc.Bacc()
    # hT[p, k*BL + m] = hidden_loc[m, k*128 + p]  (per-core batches)
    hT = nc.dram_tensor("hT", [128, KT * BL], F32, kind="ExternalInput")
    W = nc.dram_tensor("W", [H, D], F32, kind="ExternalInput")
    enc = nc.dram_tensor("enc", [BL, S, D], F32, kind="ExternalInput")
    ident = nc.dram_tensor("ident", [128, 128], F32, kind="ExternalInput")
    ones = nc.dram_tensor("ones", [1, 128], F32, kind="ExternalInput")
    out = nc.dram_tensor("out", [BL, S], F32, kind="ExternalOutput")

    with tile.TileContext(nc) as tc:
        with (
            tc.tile_pool(name="persist", bufs=1) as persist,
            tc.tile_pool(name="wpool", bufs=W_BUFS) as wpool,
            tc.tile_pool(name="vrpool", bufs=1) as vrpool,
            tc.tile_pool(name="stream", bufs=STREAM_BUFS) as stream,
            tc.tile_pool(name="psum_v", bufs=1, space="PSUM") as psum_v_pool,
            tc.tile_pool(name="psum_b", bufs=2, space="PSUM") as psum_b_pool,
            tc.tile_pool(name="psum_s", bufs=1, space="PSUM") as psum_s_pool,
        ):
            # ---- small loads first on the sync queue ----
            hT_sb = persist.tile([128, KT * BL], F32, tag="hT")
            nc.sync.dma_start(out=hT_sb, in_=hT[:, :])
            ident_sb = persist.tile([128, 128], F32, tag="ident")
            nc.sync.dma_start(out=ident_sb, in_=ident[:, :])
            ones_sb = persist.tile([1, 128], F32, tag="ones")
            nc.sync.dma_start(out=ones_sb, in_=ones[:, :])

            # ---- dummy matmul to absorb the PE sequencer's ~8 us first-
            # dispatch latency while W is still loading ----
            warm = psum_s_pool.tile([1, 128], F32, tag="tr")
            nc.tensor.matmul(
                warm, ones_sb[:, 0:1], ones_sb, start=True, stop=True
            )

            # ---- v = hidden_loc @ W -> psum [BL, D], W cycled via pool ----
            psv = psum_v_pool.tile([BL, D], F32, tag="psv")
            for k in range(KT):
                wt = wpool.tile([128, D], F32, tag="w", name=f"w{k}")
                nc.sync.dma_start(out=wt, in_=W[k * 128:(k + 1) * 128, :])
                for j in range(NJ):
                    nc.tensor.matmul(
                        psv[:, j * 512:(j + 1) * 512],
                        hT_sb[:, k * BL:(k + 1) * BL],
                        wt[:, j * 512:(j + 1) * 512],
                        start=(k == 0),
                        stop=(k == KT - 1),
                    )
            # batch 0 fast path: partition 0 of PSUM is legal for ACT, so
            # copy the v row straight out of psv -- no v_sb / SWDGE hops.
            # Chunked: copy j waits only on its own chunk's k=7 stop, so it
            # overlaps the remaining psv matmuls instead of waiting for all.
            vr0 = persist.tile([1, D], F32, tag="vr0")
            for j in range(NJ):
                nc.scalar.copy(
                    out=vr0[:, j * 512:(j + 1) * 512],
                    in_=psv[0:1, j * 512:(j + 1) * 512],
                )

            # batches 1-3 are needed much later: go via v_sb + SBUF->SBUF
            # DMA (engines can't touch partition offsets 1..3, DMA can).
            # The v_sb copy is issued after the b0 broadcast block so it
            # can't delay the DVE start on the ACT queue.
            v_sb = persist.tile([BL, D], F32, tag="vsb")

            # ---- per batch: broadcast v[b] to 128 partitions via rank-1
            # matmul ----
            v_bc = []
            for b in range(BL):
                if b == 0:
                    vr = vr0
                else:
                    if b == 1:
                        nc.scalar.copy(out=v_sb, in_=psv)
                    vr = vrpool.tile([1, D], F32, tag="vr", name=f"vr{b}")
                    nc.gpsimd.dma_start(out=vr, in_=v_sb[b:b + 1, :])
                vb = persist.tile([128, D], F32, tag=f"vb{b}", name=f"vb{b}")
                for j in range(NJ):
                    pb = psum_b_pool.tile([128, 512], F32, tag="pbc")
                    nc.tensor.matmul(
                        pb,
                        ones_sb,
                        vr[:, j * 512:(j + 1) * 512],
                        start=True,
                        stop=True,
                    )
                    # b0 alternates engines (it gates the DVE start);
                    # later batches stay off DVE so they can't stall the
                    # streaming reduction with PE-sem waits.
                    if b == 0 and j % 2 == 0:
                        nc.vector.tensor_copy(
                            out=vb[:, j * 512:(j + 1) * 512], in_=pb
                        )
                    else:
                        nc.scalar.copy(out=vb[:, j * 512:(j + 1) * 512], in_=pb)
                v_bc.append(vb)

            # ---- stream enc, fused multiply + row-reduce on DVE ----
            # s = p*SCOLS + q*SJ + j   (p = partition, column c = q*SJ + j)
            enc_r = enc[:, :, :].rearrange(
                "b (p q j) d -> b q p j d", p=128, q=NQ, j=SJ
            )
            e_tiles = [
                persist.tile([128, SCOLS], F32, tag=f"e{b}", name=f"e{b}")
                for b in range(BL)
            ]
            for b in range(BL):
                for q in range(NQ):
                    t = stream.tile([128, SJ, D], F32, tag="enc", name="enc_t")
                    # alternate HWDGE rings (SP / ACT) for descriptor-supply
                    # parallelism; both feed the same 16 SDMA engines
                    dma_eng = nc.sync if q % 2 == 0 else nc.scalar
                    dma_eng.dma_start(out=t, in_=enc_r[b, q])
                    for j in range(SJ):
                        # Fused multiply + add-reduce in one pass:
                        # out = (in0 * 1.0) * in1, accum_out = sum(out).
                        # out aliases in0 (the product is dead after the
                        # reduce).  NB: tensor_tensor_reduce wedges the device
                        # on this runtime path; scalar_tensor_tensor is the
                        # plain TENSOR_SCALAR_PTR ISA op and works.
                        nc.vector.scalar_tensor_tensor(
                            out=t[:, j, :],
                            in0=t[:, j, :],
                            scalar=1.0,
                            in1=v_bc[b],
                            op0=mybir.AluOpType.mult,
                            op1=mybir.AluOpType.mult,
                            accum_out=e_tiles[b][:, q * SJ + j:q * SJ + j + 1],
                        )

                # ---- per-batch softmax in the [128, SCOLS] layout,
                # overlapped with the next batch's streaming ----
                e = e_tiles[b]
                m_p = persist.tile([128, 1], F32, tag=f"mp{b}")
                nc.vector.tensor_reduce(
                    out=m_p, in_=e, axis=mybir.AxisListType.X,
                    op=mybir.AluOpType.max,
                )
                nm_p = persist.tile([128, 1], F32, tag=f"nmp{b}")
                nc.scalar.mul(out=nm_p, in_=m_p, mul=-1.0)
                s_p = persist.tile([128, 1], F32, tag=f"sp{b}")
                # e <- exp(e - m_p), s_p = row sums
                nc.scalar.activation(
                    out=e, in_=e,
                    func=mybir.ActivationFunctionType.Exp,
                    bias=nm_p, scale=1.0, accum_out=s_p,
                )
                # M = max_p m_p  (transpose via PE, reduce on DVE)
                mT = psum_s_pool.tile([1, 128], F32, tag="tr")
                nc.tensor.transpose(mT, m_p, ident_sb)
                mx = persist.tile([1, 1], F32, tag=f"mx{b}")
                nc.vector.tensor_reduce(
                    out=mx, in_=mT, axis=mybir.AxisListType.X,
                    op=mybir.AluOpType.max,
                )
                # -M broadcast to 128 partitions
                nmx = persist.tile([1, 1], F32, tag=f"nmx{b}")
                nc.scalar.mul(out=nmx, in_=mx, mul=-1.0)
                negMb = psum_s_pool.tile([128, 1], F32, tag="bc1")
                nc.tensor.matmul(
                    negMb, ones_sb, nmx[0:1, 0:1], start=True, stop=True
                )
                # w_p = exp(m_p - M)
                w_p = persist.tile([128, 1], F32, tag=f"wp{b}")
                nc.scalar.activation(
                    out=w_p, in_=negMb,
                    func=mybir.ActivationFunctionType.Exp,
                    bias=m_p, scale=1.0,
                )
                # Sw_p = s_p * w_p ; D = sum_p Sw_p
                sw_p = persist.tile([128, 1], F32, tag=f"swp{b}")
                nc.vector.scalar_tensor_tensor(
                    out=sw_p, in0=s_p, scalar=1.0, in1=w_p,
                    op0=mybir.AluOpType.mult, op1=mybir.AluOpType.mult,
                )
                swT = psum_s_pool.tile([1, 128], F32, tag="tr")
                nc.tensor.transpose(swT, sw_p, ident_sb)
                dsum = persist.tile([1, 1], F32, tag=f"ds{b}")
                nc.vector.tensor_reduce(
                    out=dsum, in_=swT, axis=mybir.AxisListType.X,
                    op=mybir.AluOpType.add,
                )
                rden = persist.tile([1, 1], F32, tag=f"rd{b}")
                nc.vector.reciprocal(out=rden, in_=dsum)
                rb = psum_s_pool.tile([128, 1], F32, tag="bc1")
                nc.tensor.matmul(
                    rb, ones_sb, rden[0:1, 0:1], start=True, stop=True
                )
                # f_p = w_p * (1/D) ; attn = e * f_p
                f_p = persist.tile([128, 1], F32, tag=f"fp{b}")
                nc.vector.scalar_tensor_tensor(
                    out=f_p, in0=w_p, scalar=1.0, in1=rb,
                    op0=mybir.AluOpType.mult, op1=mybir.AluOpType.mult,
                )
                nc.vector.tensor_scalar_mul(e, e, f_p)
                # out[b, p*SCOLS + c] = e[p, c]; SWDGE queue keeps the sync
                # queue pure enc streaming.
                nc.gpsimd.dma_start(out=out[b:b + 1, :], in_=e[:, :])

    nc.compile()
    return nc


_NC_CACHE = None


def _get_nc():
    global _NC_CACHE
    if _NC_CACHE is None:
        _NC_CACHE = build_bass()
    return _NC_CACHE


def _make_in_maps(hidden, encoder_outputs, W):
    hidden = np.asarray(hidden, dtype=np.float32)
    encoder_outputs = np.asarray(encoder_outputs, dtype=np.float32)
    W = np.ascontiguousarray(np.asarray(W, dtype=np.float32))
    ident = np.eye(128, dtype=np.float32)
    ones = np.ones((1, 128), np.float32)
    in_maps = []
    for c in range(NCORES):
        hs = hidden[c * BL:(c + 1) * BL]                       # [BL, H]
        hT = np.ascontiguousarray(
            hs.T.reshape(KT, 128, BL).transpose(1, 0, 2).reshape(128, KT * BL)
        )
        in_maps.append({
            "hT": hT,
            "W": W,
            "enc": np.ascontiguousarray(encoder_outputs[c * BL:(c + 1) * BL]),
            "ident": ident,
            "ones": ones,
        })
    return in_maps


def run_device(hidden, encoder_outputs, W, trace=False, **spmd_kwargs):
    nc = _get_nc()
    in_maps = _make_in_maps(hidden, encoder_outputs, W)
    res = run_bass_kernel_spmd(
        nc, in_maps, core_ids=list(range(NCORES)), trace=trace, **spmd_kwargs
    )
    outs = np.concatenate([r["out"] for r in res.results], axis=0)  # [B, S]
    return outs[:, None, :].astype(np.float32), res


def kernel(hidden, encoder_outputs, W, b):
    # `b` (the Linear bias) shifts every energy in a row equally
    # (hidden[b].bias, independent of s), so it cancels in the softmax.
    out, _ = run_device(hidden, encoder_outputs, W)
    return out
